# revision 18
# baseline (speedup 1.0000x reference)
"""Trainium2 Bass kernel for nn_MessagePassingUnit_v1 (gnn_message_passing).

Computation (per row r of N=131072):
    paired = concat(unary[r], pair[r])            # [1024]
    h = LayerNorm(paired) -> ReLU                 # [1024]
    z = h @ lin_weight.T + lin_bias               # [64]
    gate[r] = mean(sigmoid(z))                    # scalar
    output[r] = pair[r] * gate[r] * attn[r]       # [512]

Returns (output [N,512] f32, gate [N] f32).

Sharding: pure data parallel over N across 8 NeuronCores (rows split in 8
contiguous blocks); params replicated.

Per-core device pipeline (rows on partitions, 512 rows per iteration):
  DMA   : load unary/pair halves into one [128,4,1024] f32 tile + attn
  DVE   : bn_stats/bn_aggr -> mean/var; Newton-Raphson rsqrt (bit-trick seed)
  ACT   : h = Relu(x*r - mu*r) in one fused activation pass, output bf16
  DMA   : xbar transpose h -> hT (features on partitions), bf16
  PE    : z = sum_c hT_c.T @ WT_c accumulated in PSUM (bf16 x bf16 -> f32)
  ACT   : sigmoid(z) with free-dim accumulation -> sum of sigmoids per row
  DVE   : gate = accum/64 ; s = gate*attn
  GPSIMD: output = pair * s   (per-partition scalar multiply)
  DMA   : store output rows + gate
"""

import os
import sys

import numpy as np

for _p in ("/opt/trn_rl_repo", "/opt/pypackages"):
    if os.path.isdir(_p) and _p not in sys.path:
        sys.path.append(_p)

import ml_dtypes

N, D, FD = 131072, 512, 64
TWO_D = 2 * D
EPS = 1e-5
NCORES = 8
NL = N // NCORES          # rows per core
P = 128                   # partitions
G = 4                     # row-groups of 128 per iteration
RPI = P * G               # rows per iteration
NCH = TWO_D // P          # K chunks of 128 (8)
MAGIC = 0x5F3759DF        # fast inverse sqrt seed


def _emit(tc, ctx, aps, n_rows, fast):
    import concourse.bass as bass
    from concourse import mybir

    nc = tc.nc
    f32 = mybir.dt.float32
    bf16 = mybir.dt.bfloat16
    u32 = mybir.dt.uint32
    A = mybir.AluOpType
    AF = mybir.ActivationFunctionType

    unary, pair, attn, wT, lnw, lnb, linb, out, gate = aps
    nit = n_rows // RPI

    # p-major row mapping: row r = RPI*t + G*p + g, so each partition's G rows
    # are consecutive in DRAM -> 8KB-contiguous DMA descriptors.
    un_v = unary.rearrange("(t p g) d -> t p g d", p=P, g=G)
    pr_v = pair.rearrange("(t p g) d -> t p g d", p=P, g=G)
    at_v = attn.rearrange("(t p g) -> p t g", p=P, g=G)
    out_v = out.rearrange("(t p g) d -> t p g d", p=P, g=G)
    gt_v = gate.rearrange("(t p g) -> p t g", p=P, g=G)
    wt_v = wT.rearrange("(c p) f -> p c f", p=P)

    singles = ctx.enter_context(tc.tile_pool(name="singles", bufs=1))
    xupool = ctx.enter_context(tc.tile_pool(name="xu", bufs=4))
    xppool = ctx.enter_context(tc.tile_pool(name="xp", bufs=5))
    hpool = ctx.enter_context(tc.tile_pool(name="h", bufs=3))
    htpool = ctx.enter_context(tc.tile_pool(name="ht", bufs=3))
    opool = ctx.enter_context(tc.tile_pool(name="o", bufs=3))
    spool = ctx.enter_context(tc.tile_pool(name="small", bufs=4))
    sgpool = ctx.enter_context(tc.tile_pool(name="sg", bufs=4))
    pspool = ctx.enter_context(tc.tile_pool(name="ps", bufs=8, space="PSUM"))

    wt_sb = singles.tile([P, NCH, FD], bf16)
    nc.sync.dma_start(out=wt_sb[:], in_=wt_v)
    att_all = singles.tile([P, nit, G], f32)
    nc.sync.dma_start(out=att_all[:], in_=at_v)
    gate_all = singles.tile([P, nit, G], f32)

    if not fast:
        # broadcast LN affine params across partitions
        wb = singles.tile([P, TWO_D], f32)
        bb = singles.tile([P, TWO_D], f32)
        nc.gpsimd.dma_start(
            out=wb[:],
            in_=bass.AP(tensor=lnw.tensor, offset=lnw.offset, ap=[[0, P]] + list(lnw.ap)),
        )
        nc.gpsimd.dma_start(
            out=bb[:],
            in_=bass.AP(tensor=lnb.tensor, offset=lnb.offset, ap=[[0, P]] + list(lnb.ap)),
        )
        # lin_bias row for the K=1 bias matmul
        lb_sb = singles.tile([1, FD], f32)
        nc.sync.dma_start(out=lb_sb[:], in_=linb.rearrange("(o f) -> o f", o=1))
        one_sb = singles.tile([1, P], f32)
        nc.vector.memset(one_sb[:], 1.0)

    for t in range(nit):
        xu = xupool.tile([P, G, D], f32)
        xp = xppool.tile([P, G, D], f32)
        nc.gpsimd.dma_start(out=xu[:], in_=un_v[t])
        nc.gpsimd.dma_start(out=xp[:], in_=pr_v[t])

        # --- LayerNorm statistics ---
        st6 = spool.tile([P, G, 2, 6], f32)
        for g in range(G):
            nc.vector.bn_stats(out=st6[:, g, 0], in_=xu[:, g])
            nc.vector.bn_stats(out=st6[:, g, 1], in_=xp[:, g])
        mv = spool.tile([P, G, 2], f32)
        for g in range(G):
            nc.vector.bn_aggr(out=mv[:, g], in_=st6[:, g])

        # r = rsqrt(var + eps) via bit-trick seed + 3 Newton iterations (DVE only)
        u = spool.tile([P, G], f32)
        nc.vector.tensor_scalar_add(u[:], mv[:, :, 1], EPS)
        y = spool.tile([P, G], f32)
        yu = y[:].bitcast(u32)
        uu = u[:].bitcast(u32)
        # seed = bits(MAGIC - bits(u)/2), computed in fp32 value space (DVE ALU
        # is fp32 internally; |values| ~1e9 are within exact-enough fp32 range)
        nc.vector.tensor_scalar(yu, uu, -0.5, float(MAGIC), A.mult, A.add)
        yy = spool.tile([P, G], f32)
        pp = spool.tile([P, G], f32)
        hh = spool.tile([P, G], f32)
        for _ in range(2):
            nc.vector.tensor_tensor(out=yy[:], in0=y[:], in1=y[:], op=A.mult)
            nc.vector.tensor_tensor(out=pp[:], in0=yy[:], in1=u[:], op=A.mult)
            nc.vector.tensor_scalar(hh[:], pp[:], -0.5, 1.5, A.mult, A.add)
            nc.vector.tensor_tensor(out=y[:], in0=y[:], in1=hh[:], op=A.mult)
        nmr = spool.tile([P, G], f32)
        nc.vector.tensor_scalar_mul(nmr[:], mv[:, :, 0], -1.0)

        # --- normalize + relu (+ affine in general path) ---
        # fast path: h = relu(x - mu); the rsqrt scale r is folded into the
        # sigmoid's per-partition scale (r > 0 commutes with relu), which
        # keeps the Newton-rsqrt chain off the critical path.
        h = hpool.tile([P, G, TWO_D], bf16)
        if fast:
            for g in range(G):
                if g < 2:
                    nc.vector.tensor_scalar(
                        h[:, g, 0:D], xu[:, g], nmr[:, g : g + 1], 0.0, A.add, A.max
                    )
                else:
                    nc.scalar.activation(
                        out=h[:, g, 0:D],
                        in_=xu[:, g],
                        func=AF.Relu,
                        bias=nmr[:, g : g + 1],
                    )
                nc.scalar.activation(
                    out=h[:, g, D:TWO_D],
                    in_=xp[:, g],
                    func=AF.Relu,
                    bias=nmr[:, g : g + 1],
                )
        else:
            nmr2 = spool.tile([P, G], f32)
            nc.vector.tensor_tensor(out=nmr2[:], in0=nmr[:], in1=y[:], op=A.mult)
            tmp = hpool.tile([P, G, TWO_D], f32, tag="tmp_general")
            for g in range(G):
                nc.scalar.activation(
                    out=tmp[:, g, 0:D],
                    in_=xu[:, g],
                    func=AF.Identity,
                    bias=nmr2[:, g : g + 1],
                    scale=y[:, g : g + 1],
                )
                nc.scalar.activation(
                    out=tmp[:, g, D:TWO_D],
                    in_=xp[:, g],
                    func=AF.Identity,
                    bias=nmr2[:, g : g + 1],
                    scale=y[:, g : g + 1],
                )
                nc.vector.tensor_tensor(out=tmp[:, g], in0=tmp[:, g], in1=wb[:], op=A.mult)
                nc.vector.tensor_tensor(out=tmp[:, g], in0=tmp[:, g], in1=bb[:], op=A.add)
                nc.scalar.activation(out=h[:, g], in_=tmp[:, g], func=AF.Relu)

        # --- transpose h: [128 rows, (g d)] -> [128 feat, (g c), 128 rows] ---
        # split in two so matmuls for groups 0-1 can start while groups 2-3
        # are still in the relu stage
        hT = htpool.tile([P, G * NCH, P], bf16)
        half = G // 2 * NCH
        nc.sync.dma_start(
            out=hT[:, 0:half],
            in_=h[:, 0 : G // 2].rearrange("p g d -> p (g d)"),
            transpose=True,
        )
        nc.sync.dma_start(
            out=hT[:, half : G * NCH],
            in_=h[:, G // 2 : G].rearrange("p g d -> p (g d)"),
            transpose=True,
        )

        # --- matmul + sigmoid + mean ---
        gsum = spool.tile([P, G], f32)
        for g in range(G):
            ps = pspool.tile([P, FD], f32)
            for c in range(NCH):
                nc.tensor.matmul(
                    ps[:],
                    hT[:, g * NCH + c],
                    wt_sb[:, c],
                    start=(c == 0),
                    stop=(fast and c == NCH - 1),
                )
            if not fast:
                nc.tensor.matmul(ps[:], one_sb[:], lb_sb[:], start=False, stop=True)
            sg = sgpool.tile([P, FD], f32)
            if fast:
                nc.scalar.activation(
                    out=sg[:],
                    in_=ps[:],
                    func=AF.Sigmoid,
                    scale=y[:, g : g + 1],
                    accum_out=gsum[:, g : g + 1],
                )
            else:
                nc.scalar.activation(
                    out=sg[:], in_=ps[:], func=AF.Sigmoid, accum_out=gsum[:, g : g + 1]
                )

        nc.vector.tensor_scalar_mul(gate_all[:, t], gsum[:], 1.0 / FD)
        s = spool.tile([P, G], f32)
        nc.vector.tensor_tensor(
            out=s[:], in0=gate_all[:, t], in1=att_all[:, t], op=A.mult
        )

        # --- output = pair * gate * attn (mostly ACT; DVE carries bn_stats) ---
        o = opool.tile([P, G, D], f32)
        for g in range(G):
            if g == 0:
                nc.vector.tensor_scalar(o[:, g], xp[:, g], s[:, g : g + 1], None, A.mult)
            else:
                nc.scalar.activation(
                    out=o[:, g], in_=xp[:, g], func=AF.Copy, scale=s[:, g : g + 1]
                )

        nc.gpsimd.dma_start(out=out_v[t], in_=o[:])

    nc.sync.dma_start(out=gt_v, in_=gate_all[:])


def _patch_tile_drain():
    """The walrus build in this container rejects Drain instructions that
    carry sync waits ("Too many sync wait commands").  Tile's kernel-tail
    drain stacks one wait per live semaphore lane onto a single Drain.
    Re-emit those waits as individual EventSemaphore waits (which walrus
    accepts) followed by a wait-free Drain — semantically identical."""
    import bass_rust
    import concourse.tile as tile
    from concourse.vector_clock import ScopedClock

    if getattr(tile.TileContext, "_drain_waits_patched", False):
        return

    def _drain_and_barrier(self, tick_clock, wait_clock):
        nc = self.nc
        probe = nc.sync.nop()
        wait_clock.add_sem_waits(probe.ins, ScopedClock({None: tick_clock.global_clock}))
        si = probe.ins.sync_info
        waits = list(si.on_wait) if si is not None else []
        probe.ins.sync_info = bass_rust.SyncInfo(on_wait=[], on_update=[])
        name_to_handle = {}
        assert self.sems is not None
        for h in self.sems.allocated().values():
            name_to_handle[h.name] = h
        for w in waits:
            h = name_to_handle.get(w.ant_name)
            if h is not None:
                nc.sync.wait_ge(h, w.wait_value)
            else:
                ev = nc.sync.nop()
                ev.ins.sync_info = bass_rust.SyncInfo(on_wait=[w], on_update=[])
        nc.sync.drain()
        nc.all_engine_barrier()
        popped = nc._tile_sem_poison_stack.pop()
        assert popped is self._sem_poison
        nc.clear_and_free_semaphores(list(self.sems.allocated().values()))
        nc.all_engine_barrier()

    tile.TileContext._drain_and_barrier = _drain_and_barrier
    tile.TileContext._drain_waits_patched = True


MAX_INST_WAITS = 2


def _patch_tile_wait_split():
    """Same walrus limitation as _patch_tile_drain, applied generally: any
    instruction carrying more than MAX_INST_WAITS sync waits gets the excess
    hoisted into standalone single-wait EventSemaphore instructions emitted
    just before it on the same engine."""
    import bass_rust
    import concourse.tile as tile
    from concourse import mybir

    if getattr(tile.TileContext, "_wait_split_patched", False):
        return
    orig = tile.TileContext._lower_ordered_insts

    def patched(self, ordered):
        nid = 0
        for bbname, insts in ordered.items():
            out = []
            for inst in insts:
                si = inst.sync_info
                waits = list(si.on_wait) if si else []
                n_up = len(si.on_update) if si else 0
                limit = max(0, MAX_INST_WAITS - n_up)
                if str(inst.opcode) == "Drain":
                    limit = 0
                if len(waits) > limit:
                    keep = waits[len(waits) - limit :] if limit else []
                    for w in waits[: len(waits) - limit]:
                        nid += 1
                        ev = mybir.InstEventSemaphore(name=f"WS{nid}-{inst.name}")
                        ev.engine = inst.engine
                        ev.sync_info = bass_rust.SyncInfo(on_wait=[w], on_update=[])
                        out.append(ev)
                    inst.sync_info = bass_rust.SyncInfo(
                        on_wait=keep, on_update=list(si.on_update) if si else []
                    )
                out.append(inst)
            ordered[bbname] = out
        return orig(self, ordered)

    tile.TileContext._lower_ordered_insts = patched
    tile.TileContext._wait_split_patched = True


def build_nc(n_rows=NL, fast=True):
    """Build the per-core Bass program (SPMD: same program on every core)."""
    from contextlib import ExitStack

    import concourse.bass as bass
    import concourse.tile as tile
    from concourse import mybir

    _patch_tile_drain()
    _patch_tile_wait_split()

    f32 = mybir.dt.float32
    bf16 = mybir.dt.bfloat16

    nc = bass.Bass("TRN2", target_bir_lowering=False, debug=False, num_devices=NCORES)
    unary = nc.dram_tensor("unary", [n_rows, D], f32, kind="ExternalInput").ap()
    pair = nc.dram_tensor("pair", [n_rows, D], f32, kind="ExternalInput").ap()
    attn = nc.dram_tensor("attn", [n_rows], f32, kind="ExternalInput").ap()
    wT = nc.dram_tensor("wT", [TWO_D, FD], bf16, kind="ExternalInput").ap()
    lnw = nc.dram_tensor("lnw", [TWO_D], f32, kind="ExternalInput").ap()
    lnb = nc.dram_tensor("lnb", [TWO_D], f32, kind="ExternalInput").ap()
    linb = nc.dram_tensor("linb", [FD], f32, kind="ExternalInput").ap()
    out = nc.dram_tensor("output", [n_rows, D], f32, kind="ExternalOutput").ap()
    gate = nc.dram_tensor("gate", [n_rows], f32, kind="ExternalOutput").ap()

    aps = (unary, pair, attn, wT, lnw, lnb, linb, out, gate)
    with tile.TileContext(nc) as tc:
        with ExitStack() as ctx:
            _emit(tc, ctx, aps, n_rows, fast)
    return nc


_CACHE = {}


def _get_nc(fast):
    key = ("nc", fast)
    if key not in _CACHE:
        _CACHE[key] = build_nc(NL, fast)
    return _CACHE[key]


def make_in_maps(unary_term, pair_term, attn_value, ln_weight, ln_bias,
                 lin_weight, lin_bias):
    wTb = np.ascontiguousarray(np.asarray(lin_weight).T).astype(ml_dtypes.bfloat16)
    unary_term = np.ascontiguousarray(np.asarray(unary_term, dtype=np.float32))
    pair_term = np.ascontiguousarray(np.asarray(pair_term, dtype=np.float32))
    attn_value = np.ascontiguousarray(np.asarray(attn_value, dtype=np.float32))
    common = {
        "wT": wTb,
        "lnw": np.asarray(ln_weight, dtype=np.float32),
        "lnb": np.asarray(ln_bias, dtype=np.float32),
        "linb": np.asarray(lin_bias, dtype=np.float32),
    }
    in_maps = []
    for i in range(NCORES):
        sl = slice(i * NL, (i + 1) * NL)
        in_maps.append(
            {
                "unary": unary_term[sl],
                "pair": pair_term[sl],
                "attn": attn_value[sl],
                **common,
            }
        )
    return in_maps


def kernel(unary_term, pair_term, attn_value, ln_weight, ln_bias,
           lin_weight, lin_bias):
    from concourse.bass_utils import run_bass_kernel_spmd

    fast = (
        np.allclose(np.asarray(ln_weight), 1.0)
        and np.allclose(np.asarray(ln_bias), 0.0)
        and np.allclose(np.asarray(lin_bias), 0.0)
    )
    nc = _get_nc(fast)
    in_maps = make_in_maps(
        unary_term, pair_term, attn_value, ln_weight, ln_bias, lin_weight, lin_bias
    )
    res = run_bass_kernel_spmd(nc, in_maps, list(range(NCORES)))
    output = np.concatenate([r["output"] for r in res.results], axis=0)
    gate = np.concatenate([r["gate"] for r in res.results], axis=0)
    return output.astype(np.float32), gate.astype(np.float32)


# revision 23
# speedup vs baseline: 1.2211x; 1.2211x over previous
"""Trainium2 Bass kernel for nn_MessagePassingUnit_v1 (gnn_message_passing).

Computation (per row r of N=131072):
    paired = concat(unary[r], pair[r])            # [1024]
    h = LayerNorm(paired) -> ReLU                 # [1024]
    z = h @ lin_weight.T + lin_bias               # [64]
    gate[r] = mean(sigmoid(z))                    # scalar
    output[r] = pair[r] * gate[r] * attn[r]       # [512]

Returns (output [N,512] f32, gate [N] f32).

Sharding: pure data parallel over N across 8 NeuronCores (rows split in 8
contiguous blocks); params replicated.

Per-core device pipeline (rows on partitions, 512 rows per iteration):
  DMA   : load unary/pair halves into one [128,4,1024] f32 tile + attn
  DVE   : bn_stats/bn_aggr -> mean/var; Newton-Raphson rsqrt (bit-trick seed)
  ACT   : h = Relu(x*r - mu*r) in one fused activation pass, output bf16
  DMA   : xbar transpose h -> hT (features on partitions), bf16
  PE    : z = sum_c hT_c.T @ WT_c accumulated in PSUM (bf16 x bf16 -> f32)
  ACT   : sigmoid(z) with free-dim accumulation -> sum of sigmoids per row
  DVE   : gate = accum/64 ; s = gate*attn
  GPSIMD: output = pair * s   (per-partition scalar multiply)
  DMA   : store output rows + gate
"""

import os
import sys

import numpy as np

for _p in ("/opt/trn_rl_repo", "/opt/pypackages"):
    if os.path.isdir(_p) and _p not in sys.path:
        sys.path.append(_p)

import ml_dtypes

N, D, FD = 131072, 512, 64
TWO_D = 2 * D
EPS = 1e-5
NCORES = 8
NL = N // NCORES          # rows per core
P = 128                   # partitions
G = 4                     # row-groups of 128 per iteration
RPI = P * G               # rows per iteration
NCH = TWO_D // P          # K chunks of 128 (8)
MAGIC = 0x5F3759DF        # fast inverse sqrt seed


def _emit(tc, ctx, aps, n_rows, fast):
    import concourse.bass as bass
    from concourse import mybir

    nc = tc.nc
    f32 = mybir.dt.float32
    bf16 = mybir.dt.bfloat16
    u32 = mybir.dt.uint32
    A = mybir.AluOpType
    AF = mybir.ActivationFunctionType

    unary, pair, attn, wT, lnw, lnb, linb, out, gate = aps
    nit = n_rows // RPI

    # p-major row mapping: row r = RPI*t + G*p + g, so each partition's G rows
    # are consecutive in DRAM -> 8KB-contiguous DMA descriptors.
    un_v = unary.rearrange("(t p g) d -> t p g d", p=P, g=G)
    pr_v = pair.rearrange("(t p g) d -> t p g d", p=P, g=G)
    at_v = attn.rearrange("(t p g) -> p t g", p=P, g=G)
    out_v = out.rearrange("(t p g) d -> t p g d", p=P, g=G)
    gt_v = gate.rearrange("(t p g) -> p t g", p=P, g=G)
    wt_v = wT.rearrange("(c p) f -> p c f", p=P)

    singles = ctx.enter_context(tc.tile_pool(name="singles", bufs=1))
    xupool = ctx.enter_context(tc.tile_pool(name="xu", bufs=4))
    xppool = ctx.enter_context(tc.tile_pool(name="xp", bufs=6))
    hpool = ctx.enter_context(tc.tile_pool(name="h", bufs=3))
    htpool = ctx.enter_context(tc.tile_pool(name="ht", bufs=3))
    opool = ctx.enter_context(tc.tile_pool(name="o", bufs=3))
    spool = ctx.enter_context(tc.tile_pool(name="small", bufs=4))
    sgpool = ctx.enter_context(tc.tile_pool(name="sg", bufs=4))
    pspool = ctx.enter_context(tc.tile_pool(name="ps", bufs=8, space="PSUM"))

    wt_sb = singles.tile([P, NCH, FD], bf16)
    nc.sync.dma_start(out=wt_sb[:], in_=wt_v)
    att_all = singles.tile([P, nit, G], f32)
    nc.sync.dma_start(out=att_all[:], in_=at_v)
    gate_all = singles.tile([P, nit, G], f32)

    if not fast:
        # broadcast LN affine params across partitions
        wb = singles.tile([P, TWO_D], f32)
        bb = singles.tile([P, TWO_D], f32)
        nc.gpsimd.dma_start(
            out=wb[:],
            in_=bass.AP(tensor=lnw.tensor, offset=lnw.offset, ap=[[0, P]] + list(lnw.ap)),
        )
        nc.gpsimd.dma_start(
            out=bb[:],
            in_=bass.AP(tensor=lnb.tensor, offset=lnb.offset, ap=[[0, P]] + list(lnb.ap)),
        )
        # lin_bias row for the K=1 bias matmul
        lb_sb = singles.tile([1, FD], f32)
        nc.sync.dma_start(out=lb_sb[:], in_=linb.rearrange("(o f) -> o f", o=1))
        one_sb = singles.tile([1, P], f32)
        nc.vector.memset(one_sb[:], 1.0)

    # Software-pipelined emission: the per-iteration dependency chain
    # (load -> stats -> relu -> transpose -> matmul -> sigmoid -> scale ->
    # store) is ~15-20us long while each engine only has ~8-10us of work per
    # iteration.  Each engine executes its queue in order, so emitting whole
    # iterations back-to-back serializes on the chain latency.  Emit in
    # skewed stages instead so every engine's stream interleaves several
    # iterations.
    st = {}

    def s0_load(t):
        xu = xupool.tile([P, G, D], f32)
        xp = xppool.tile([P, G, D], f32)
        nc.gpsimd.dma_start(out=xu[:], in_=un_v[t])
        nc.gpsimd.dma_start(out=xp[:], in_=pr_v[t])
        st[t] = {"xu": xu, "xp": xp}

    def s1_stats(t):
        v = st[t]
        xu, xp = v["xu"], v["xp"]
        st6 = spool.tile([P, G, 2, 6], f32)
        for g in range(G):
            nc.vector.bn_stats(out=st6[:, g, 0], in_=xu[:, g])
            nc.vector.bn_stats(out=st6[:, g, 1], in_=xp[:, g])
        mv = spool.tile([P, G, 2], f32)
        for g in range(G):
            nc.vector.bn_aggr(out=mv[:, g], in_=st6[:, g])

        # r = rsqrt(var + eps): bit-trick seed + 2 Newton iterations (DVE only)
        u = spool.tile([P, G], f32)
        nc.vector.tensor_scalar_add(u[:], mv[:, :, 1], EPS)
        y = spool.tile([P, G], f32)
        yu = y[:].bitcast(u32)
        uu = u[:].bitcast(u32)
        nc.vector.tensor_scalar(yu, uu, -0.5, float(MAGIC), A.mult, A.add)
        yy = spool.tile([P, G], f32)
        pp = spool.tile([P, G], f32)
        hh = spool.tile([P, G], f32)
        for _ in range(2):
            nc.vector.tensor_tensor(out=yy[:], in0=y[:], in1=y[:], op=A.mult)
            nc.vector.tensor_tensor(out=pp[:], in0=yy[:], in1=u[:], op=A.mult)
            nc.vector.tensor_scalar(hh[:], pp[:], -0.5, 1.5, A.mult, A.add)
            nc.vector.tensor_tensor(out=y[:], in0=y[:], in1=hh[:], op=A.mult)
        nmr = spool.tile([P, G], f32)
        nc.vector.tensor_scalar_mul(nmr[:], mv[:, :, 0], -1.0)
        v["y"] = y
        v["nmr"] = nmr

    def s2_relu(t):
        v = st[t]
        xu, xp, y, nmr = v["xu"], v["xp"], v["y"], v["nmr"]
        # fast path: h = relu(x - mu); the rsqrt scale r is folded into the
        # sigmoid's per-partition scale (r > 0 commutes with relu), keeping
        # the Newton-rsqrt chain off the critical path.
        h = hpool.tile([P, G, TWO_D], bf16)
        if fast:
            for g in range(G):
                if g < 2:
                    nc.vector.tensor_scalar(
                        h[:, g, 0:D], xu[:, g], nmr[:, g : g + 1], 0.0, A.add, A.max
                    )
                else:
                    nc.scalar.activation(
                        out=h[:, g, 0:D],
                        in_=xu[:, g],
                        func=AF.Relu,
                        bias=nmr[:, g : g + 1],
                    )
                nc.scalar.activation(
                    out=h[:, g, D:TWO_D],
                    in_=xp[:, g],
                    func=AF.Relu,
                    bias=nmr[:, g : g + 1],
                )
        else:
            nmr2 = spool.tile([P, G], f32)
            nc.vector.tensor_tensor(out=nmr2[:], in0=nmr[:], in1=y[:], op=A.mult)
            tmp = hpool.tile([P, G, TWO_D], f32, tag="tmp_general")
            for g in range(G):
                nc.scalar.activation(
                    out=tmp[:, g, 0:D],
                    in_=xu[:, g],
                    func=AF.Identity,
                    bias=nmr2[:, g : g + 1],
                    scale=y[:, g : g + 1],
                )
                nc.scalar.activation(
                    out=tmp[:, g, D:TWO_D],
                    in_=xp[:, g],
                    func=AF.Identity,
                    bias=nmr2[:, g : g + 1],
                    scale=y[:, g : g + 1],
                )
                nc.vector.tensor_tensor(out=tmp[:, g], in0=tmp[:, g], in1=wb[:], op=A.mult)
                nc.vector.tensor_tensor(out=tmp[:, g], in0=tmp[:, g], in1=bb[:], op=A.add)
                nc.scalar.activation(out=h[:, g], in_=tmp[:, g], func=AF.Relu)

        # transpose h: [128 rows, (g d)] -> [128 feat, (g c), 128 rows]
        hT = htpool.tile([P, G * NCH, P], bf16)
        half = G // 2 * NCH
        nc.sync.dma_start(
            out=hT[:, 0:half],
            in_=h[:, 0 : G // 2].rearrange("p g d -> p (g d)"),
            transpose=True,
        )
        nc.sync.dma_start(
            out=hT[:, half : G * NCH],
            in_=h[:, G // 2 : G].rearrange("p g d -> p (g d)"),
            transpose=True,
        )
        v["hT"] = hT

    def s3_matmul(t):
        v = st[t]
        hT, y = v["hT"], v["y"]
        sg = sgpool.tile([P, G, FD], f32)
        for g in range(G):
            ps = pspool.tile([P, FD], f32)
            for c in range(NCH):
                nc.tensor.matmul(
                    ps[:],
                    hT[:, g * NCH + c],
                    wt_sb[:, c],
                    start=(c == 0),
                    stop=(fast and c == NCH - 1),
                )
            if not fast:
                nc.tensor.matmul(ps[:], one_sb[:], lb_sb[:], start=False, stop=True)
            if fast:
                nc.scalar.activation(
                    out=sg[:, g], in_=ps[:], func=AF.Sigmoid, scale=y[:, g : g + 1]
                )
            else:
                nc.scalar.activation(out=sg[:, g], in_=ps[:], func=AF.Sigmoid)
        gsum = spool.tile([P, G], f32)
        nc.vector.reduce_sum(out=gsum[:], in_=sg[:], axis=mybir.AxisListType.X)
        nc.vector.tensor_scalar_mul(gate_all[:, t], gsum[:], 1.0 / FD)
        s = spool.tile([P, G], f32)
        nc.vector.tensor_tensor(
            out=s[:], in0=gate_all[:, t], in1=att_all[:, t], op=A.mult
        )
        v["s"] = s

    def s4_out(t):
        v = st.pop(t)
        xp, s = v["xp"], v["s"]
        o = opool.tile([P, G, D], f32)
        for g in range(G):
            if g == 0:
                nc.vector.tensor_scalar(o[:, g], xp[:, g], s[:, g : g + 1], None, A.mult)
            else:
                nc.scalar.activation(
                    out=o[:, g], in_=xp[:, g], func=AF.Copy, scale=s[:, g : g + 1]
                )
        # SWDGE store: keeps SP free for transposes (mixing DMACopy with
        # DMATranspose on SP forces xbar-mode serialization), and the skew
        # keeps this store from head-of-line-blocking later loads
        nc.gpsimd.dma_start(out=out_v[t], in_=o[:])

    stages = (s0_load, s1_stats, s2_relu, s3_matmul, s4_out)
    nstage = len(stages)
    for u in range(nit + nstage - 1):
        for k, stage in enumerate(stages):
            t = u - k
            if 0 <= t < nit:
                stage(t)

    nc.sync.dma_start(out=gt_v, in_=gate_all[:])


def _patch_tile_drain():
    """The walrus build in this container rejects Drain instructions that
    carry sync waits ("Too many sync wait commands").  Tile's kernel-tail
    drain stacks one wait per live semaphore lane onto a single Drain.
    Re-emit those waits as individual EventSemaphore waits (which walrus
    accepts) followed by a wait-free Drain — semantically identical."""
    import bass_rust
    import concourse.tile as tile
    from concourse.vector_clock import ScopedClock

    if getattr(tile.TileContext, "_drain_waits_patched", False):
        return

    def _drain_and_barrier(self, tick_clock, wait_clock):
        nc = self.nc
        probe = nc.sync.nop()
        wait_clock.add_sem_waits(probe.ins, ScopedClock({None: tick_clock.global_clock}))
        si = probe.ins.sync_info
        waits = list(si.on_wait) if si is not None else []
        probe.ins.sync_info = bass_rust.SyncInfo(on_wait=[], on_update=[])
        name_to_handle = {}
        assert self.sems is not None
        for h in self.sems.allocated().values():
            name_to_handle[h.name] = h
        for w in waits:
            h = name_to_handle.get(w.ant_name)
            if h is not None:
                nc.sync.wait_ge(h, w.wait_value)
            else:
                ev = nc.sync.nop()
                ev.ins.sync_info = bass_rust.SyncInfo(on_wait=[w], on_update=[])
        nc.sync.drain()
        nc.all_engine_barrier()
        popped = nc._tile_sem_poison_stack.pop()
        assert popped is self._sem_poison
        nc.clear_and_free_semaphores(list(self.sems.allocated().values()))
        nc.all_engine_barrier()

    tile.TileContext._drain_and_barrier = _drain_and_barrier
    tile.TileContext._drain_waits_patched = True


MAX_INST_WAITS = 2


def _patch_tile_wait_split():
    """Same walrus limitation as _patch_tile_drain, applied generally: any
    instruction carrying more than MAX_INST_WAITS sync waits gets the excess
    hoisted into standalone single-wait EventSemaphore instructions emitted
    just before it on the same engine."""
    import bass_rust
    import concourse.tile as tile
    from concourse import mybir

    if getattr(tile.TileContext, "_wait_split_patched", False):
        return
    orig = tile.TileContext._lower_ordered_insts

    def patched(self, ordered):
        nid = 0
        for bbname, insts in ordered.items():
            out = []
            for inst in insts:
                si = inst.sync_info
                waits = list(si.on_wait) if si else []
                n_up = len(si.on_update) if si else 0
                limit = max(0, MAX_INST_WAITS - n_up)
                if str(inst.opcode) == "Drain":
                    limit = 0
                if len(waits) > limit:
                    keep = waits[len(waits) - limit :] if limit else []
                    for w in waits[: len(waits) - limit]:
                        nid += 1
                        ev = mybir.InstEventSemaphore(name=f"WS{nid}-{inst.name}")
                        ev.engine = inst.engine
                        ev.sync_info = bass_rust.SyncInfo(on_wait=[w], on_update=[])
                        out.append(ev)
                    inst.sync_info = bass_rust.SyncInfo(
                        on_wait=keep, on_update=list(si.on_update) if si else []
                    )
                out.append(inst)
            ordered[bbname] = out
        return orig(self, ordered)

    tile.TileContext._lower_ordered_insts = patched
    tile.TileContext._wait_split_patched = True


def build_nc(n_rows=NL, fast=True):
    """Build the per-core Bass program (SPMD: same program on every core)."""
    from contextlib import ExitStack

    import concourse.bass as bass
    import concourse.tile as tile
    from concourse import mybir

    _patch_tile_drain()
    _patch_tile_wait_split()

    f32 = mybir.dt.float32
    bf16 = mybir.dt.bfloat16

    nc = bass.Bass("TRN2", target_bir_lowering=False, debug=False, num_devices=NCORES)
    unary = nc.dram_tensor("unary", [n_rows, D], f32, kind="ExternalInput").ap()
    pair = nc.dram_tensor("pair", [n_rows, D], f32, kind="ExternalInput").ap()
    attn = nc.dram_tensor("attn", [n_rows], f32, kind="ExternalInput").ap()
    wT = nc.dram_tensor("wT", [TWO_D, FD], bf16, kind="ExternalInput").ap()
    lnw = nc.dram_tensor("lnw", [TWO_D], f32, kind="ExternalInput").ap()
    lnb = nc.dram_tensor("lnb", [TWO_D], f32, kind="ExternalInput").ap()
    linb = nc.dram_tensor("linb", [FD], f32, kind="ExternalInput").ap()
    out = nc.dram_tensor("output", [n_rows, D], f32, kind="ExternalOutput").ap()
    gate = nc.dram_tensor("gate", [n_rows], f32, kind="ExternalOutput").ap()

    aps = (unary, pair, attn, wT, lnw, lnb, linb, out, gate)
    with tile.TileContext(nc) as tc:
        with ExitStack() as ctx:
            _emit(tc, ctx, aps, n_rows, fast)
    return nc


_CACHE = {}


def _get_nc(fast):
    key = ("nc", fast)
    if key not in _CACHE:
        _CACHE[key] = build_nc(NL, fast)
    return _CACHE[key]


def make_in_maps(unary_term, pair_term, attn_value, ln_weight, ln_bias,
                 lin_weight, lin_bias):
    wTb = np.ascontiguousarray(np.asarray(lin_weight).T).astype(ml_dtypes.bfloat16)
    unary_term = np.ascontiguousarray(np.asarray(unary_term, dtype=np.float32))
    pair_term = np.ascontiguousarray(np.asarray(pair_term, dtype=np.float32))
    attn_value = np.ascontiguousarray(np.asarray(attn_value, dtype=np.float32))
    common = {
        "wT": wTb,
        "lnw": np.asarray(ln_weight, dtype=np.float32),
        "lnb": np.asarray(ln_bias, dtype=np.float32),
        "linb": np.asarray(lin_bias, dtype=np.float32),
    }
    in_maps = []
    for i in range(NCORES):
        sl = slice(i * NL, (i + 1) * NL)
        in_maps.append(
            {
                "unary": unary_term[sl],
                "pair": pair_term[sl],
                "attn": attn_value[sl],
                **common,
            }
        )
    return in_maps


def kernel(unary_term, pair_term, attn_value, ln_weight, ln_bias,
           lin_weight, lin_bias):
    from concourse.bass_utils import run_bass_kernel_spmd

    fast = (
        np.allclose(np.asarray(ln_weight), 1.0)
        and np.allclose(np.asarray(ln_bias), 0.0)
        and np.allclose(np.asarray(lin_bias), 0.0)
    )
    nc = _get_nc(fast)
    in_maps = make_in_maps(
        unary_term, pair_term, attn_value, ln_weight, ln_bias, lin_weight, lin_bias
    )
    res = run_bass_kernel_spmd(nc, in_maps, list(range(NCORES)))
    output = np.concatenate([r["output"] for r in res.results], axis=0)
    gate = np.concatenate([r["gate"] for r in res.results], axis=0)
    return output.astype(np.float32), gate.astype(np.float32)


# revision 26
# speedup vs baseline: 1.2253x; 1.0034x over previous
"""Trainium2 Bass kernel for nn_MessagePassingUnit_v1 (gnn_message_passing).

Computation (per row r of N=131072):
    paired = concat(unary[r], pair[r])            # [1024]
    h = LayerNorm(paired) -> ReLU                 # [1024]
    z = h @ lin_weight.T + lin_bias               # [64]
    gate[r] = mean(sigmoid(z))                    # scalar
    output[r] = pair[r] * gate[r] * attn[r]       # [512]

Returns (output [N,512] f32, gate [N] f32).

Sharding: pure data parallel over N across 8 NeuronCores (rows split in 8
contiguous blocks); params replicated.

Per-core device pipeline (rows on partitions, 512 rows per iteration):
  DMA   : load unary/pair halves into one [128,4,1024] f32 tile + attn
  DVE   : bn_stats/bn_aggr -> mean/var; Newton-Raphson rsqrt (bit-trick seed)
  ACT   : h = Relu(x*r - mu*r) in one fused activation pass, output bf16
  DMA   : xbar transpose h -> hT (features on partitions), bf16
  PE    : z = sum_c hT_c.T @ WT_c accumulated in PSUM (bf16 x bf16 -> f32)
  ACT   : sigmoid(z) with free-dim accumulation -> sum of sigmoids per row
  DVE   : gate = accum/64 ; s = gate*attn
  GPSIMD: output = pair * s   (per-partition scalar multiply)
  DMA   : store output rows + gate
"""

import os
import sys

import numpy as np

for _p in ("/opt/trn_rl_repo", "/opt/pypackages"):
    if os.path.isdir(_p) and _p not in sys.path:
        sys.path.append(_p)

import ml_dtypes

N, D, FD = 131072, 512, 64
TWO_D = 2 * D
EPS = 1e-5
NCORES = 8
NL = N // NCORES          # rows per core
P = 128                   # partitions
G = 4                     # row-groups of 128 per iteration
RPI = P * G               # rows per iteration
NCH = TWO_D // P          # K chunks of 128 (8)
MAGIC = 0x5F3759DF        # fast inverse sqrt seed


def _emit(tc, ctx, aps, n_rows, fast):
    import concourse.bass as bass
    from concourse import mybir

    nc = tc.nc
    f32 = mybir.dt.float32
    bf16 = mybir.dt.bfloat16
    u32 = mybir.dt.uint32
    A = mybir.AluOpType
    AF = mybir.ActivationFunctionType

    unary, pair, attn, wT, lnw, lnb, linb, out, gate = aps
    nit = n_rows // RPI

    # p-major row mapping: row r = RPI*t + G*p + g, so each partition's G rows
    # are consecutive in DRAM -> 8KB-contiguous DMA descriptors.
    un_v = unary.rearrange("(t p g) d -> t p g d", p=P, g=G)
    pr_v = pair.rearrange("(t p g) d -> t p g d", p=P, g=G)
    at_v = attn.rearrange("(t p g) -> p t g", p=P, g=G)
    out_v = out.rearrange("(t p g) d -> t p g d", p=P, g=G)
    gt_v = gate.rearrange("(t p g) -> p t g", p=P, g=G)
    wt_v = wT.rearrange("(c p) f -> p c f", p=P)

    singles = ctx.enter_context(tc.tile_pool(name="singles", bufs=1))
    xupool = ctx.enter_context(tc.tile_pool(name="xu", bufs=4))
    xppool = ctx.enter_context(tc.tile_pool(name="xp", bufs=7))
    hpool = ctx.enter_context(tc.tile_pool(name="h", bufs=3))
    htpool = ctx.enter_context(tc.tile_pool(name="ht", bufs=3))
    opool = ctx.enter_context(tc.tile_pool(name="o", bufs=3))
    spool = ctx.enter_context(tc.tile_pool(name="small", bufs=6))
    sgpool = ctx.enter_context(tc.tile_pool(name="sg", bufs=4))
    pspool = ctx.enter_context(tc.tile_pool(name="ps", bufs=8, space="PSUM"))

    wt_sb = singles.tile([P, NCH, FD], bf16)
    nc.sync.dma_start(out=wt_sb[:], in_=wt_v)
    att_all = singles.tile([P, nit, G], f32)
    nc.sync.dma_start(out=att_all[:], in_=at_v)
    gate_all = singles.tile([P, nit, G], f32)

    if not fast:
        # broadcast LN affine params across partitions
        wb = singles.tile([P, TWO_D], f32)
        bb = singles.tile([P, TWO_D], f32)
        nc.gpsimd.dma_start(
            out=wb[:],
            in_=bass.AP(tensor=lnw.tensor, offset=lnw.offset, ap=[[0, P]] + list(lnw.ap)),
        )
        nc.gpsimd.dma_start(
            out=bb[:],
            in_=bass.AP(tensor=lnb.tensor, offset=lnb.offset, ap=[[0, P]] + list(lnb.ap)),
        )
        # lin_bias row for the K=1 bias matmul
        lb_sb = singles.tile([1, FD], f32)
        nc.sync.dma_start(out=lb_sb[:], in_=linb.rearrange("(o f) -> o f", o=1))
        one_sb = singles.tile([1, P], f32)
        nc.vector.memset(one_sb[:], 1.0)

    # Software-pipelined emission: the per-iteration dependency chain
    # (load -> stats -> relu -> transpose -> matmul -> sigmoid -> scale ->
    # store) is ~15-20us long while each engine only has ~8-10us of work per
    # iteration.  Each engine executes its queue in order, so emitting whole
    # iterations back-to-back serializes on the chain latency.  Emit in
    # skewed stages instead so every engine's stream interleaves several
    # iterations.
    st = {}

    def s0_load(t):
        xu = xupool.tile([P, G, D], f32)
        xp = xppool.tile([P, G, D], f32)
        nc.gpsimd.dma_start(out=xu[:], in_=un_v[t])
        nc.gpsimd.dma_start(out=xp[:], in_=pr_v[t])
        st[t] = {"xu": xu, "xp": xp}

    def s1_stats(t):
        v = st[t]
        xu, xp = v["xu"], v["xp"]
        st6 = spool.tile([P, G, 2, 6], f32)
        for g in range(G):
            nc.vector.bn_stats(out=st6[:, g, 0], in_=xu[:, g])
            nc.vector.bn_stats(out=st6[:, g, 1], in_=xp[:, g])
        mv = spool.tile([P, G, 2], f32)
        for g in range(G):
            nc.vector.bn_aggr(out=mv[:, g], in_=st6[:, g])

        # r = rsqrt(var + eps): bit-trick seed + 2 Newton iterations (DVE only)
        u = spool.tile([P, G], f32)
        nc.vector.tensor_scalar_add(u[:], mv[:, :, 1], EPS)
        y = spool.tile([P, G], f32)
        yu = y[:].bitcast(u32)
        uu = u[:].bitcast(u32)
        nc.vector.tensor_scalar(yu, uu, -0.5, float(MAGIC), A.mult, A.add)
        yy = spool.tile([P, G], f32)
        pp = spool.tile([P, G], f32)
        hh = spool.tile([P, G], f32)
        for _ in range(2):
            nc.vector.tensor_tensor(out=yy[:], in0=y[:], in1=y[:], op=A.mult)
            nc.vector.tensor_tensor(out=pp[:], in0=yy[:], in1=u[:], op=A.mult)
            nc.vector.tensor_scalar(hh[:], pp[:], -0.5, 1.5, A.mult, A.add)
            nc.vector.tensor_tensor(out=y[:], in0=y[:], in1=hh[:], op=A.mult)
        nmr = spool.tile([P, G], f32)
        nc.vector.tensor_scalar_mul(nmr[:], mv[:, :, 0], -1.0)
        v["y"] = y
        v["nmr"] = nmr

    def s2_relu(t):
        v = st[t]
        xu, xp, y, nmr = v["xu"], v["xp"], v["y"], v["nmr"]
        # fast path: h = relu(x - mu); the rsqrt scale r is folded into the
        # sigmoid's per-partition scale (r > 0 commutes with relu), keeping
        # the Newton-rsqrt chain off the critical path.
        h = hpool.tile([P, G, TWO_D], bf16)
        if fast:
            for g in range(G):
                if g < 2:
                    nc.vector.tensor_scalar(
                        h[:, g, 0:D], xu[:, g], nmr[:, g : g + 1], 0.0, A.add, A.max
                    )
                else:
                    nc.scalar.activation(
                        out=h[:, g, 0:D],
                        in_=xu[:, g],
                        func=AF.Relu,
                        bias=nmr[:, g : g + 1],
                    )
                nc.scalar.activation(
                    out=h[:, g, D:TWO_D],
                    in_=xp[:, g],
                    func=AF.Relu,
                    bias=nmr[:, g : g + 1],
                )
        else:
            nmr2 = spool.tile([P, G], f32)
            nc.vector.tensor_tensor(out=nmr2[:], in0=nmr[:], in1=y[:], op=A.mult)
            tmp = hpool.tile([P, G, TWO_D], f32, tag="tmp_general")
            for g in range(G):
                nc.scalar.activation(
                    out=tmp[:, g, 0:D],
                    in_=xu[:, g],
                    func=AF.Identity,
                    bias=nmr2[:, g : g + 1],
                    scale=y[:, g : g + 1],
                )
                nc.scalar.activation(
                    out=tmp[:, g, D:TWO_D],
                    in_=xp[:, g],
                    func=AF.Identity,
                    bias=nmr2[:, g : g + 1],
                    scale=y[:, g : g + 1],
                )
                nc.vector.tensor_tensor(out=tmp[:, g], in0=tmp[:, g], in1=wb[:], op=A.mult)
                nc.vector.tensor_tensor(out=tmp[:, g], in0=tmp[:, g], in1=bb[:], op=A.add)
                nc.scalar.activation(out=h[:, g], in_=tmp[:, g], func=AF.Relu)

        # transpose h: [128 rows, (g d)] -> [128 feat, (g c), 128 rows]
        hT = htpool.tile([P, G * NCH, P], bf16)
        half = G // 2 * NCH
        nc.sync.dma_start(
            out=hT[:, 0:half],
            in_=h[:, 0 : G // 2].rearrange("p g d -> p (g d)"),
            transpose=True,
        )
        nc.sync.dma_start(
            out=hT[:, half : G * NCH],
            in_=h[:, G // 2 : G].rearrange("p g d -> p (g d)"),
            transpose=True,
        )
        v["hT"] = hT

    def s3_matmul(t):
        v = st[t]
        hT = v["hT"]
        pss = []
        for g in range(G):
            ps = pspool.tile([P, FD], f32)
            for c in range(NCH):
                nc.tensor.matmul(
                    ps[:],
                    hT[:, g * NCH + c],
                    wt_sb[:, c],
                    start=(c == 0),
                    stop=(fast and c == NCH - 1),
                )
            if not fast:
                nc.tensor.matmul(ps[:], one_sb[:], lb_sb[:], start=False, stop=True)
            pss.append(ps)
        v["pss"] = pss

    def s4_sigmoid(t):
        v = st[t]
        y = v["y"]
        sg = sgpool.tile([P, G, FD], f32)
        for g, ps in enumerate(v.pop("pss")):
            if fast:
                nc.scalar.activation(
                    out=sg[:, g], in_=ps[:], func=AF.Sigmoid, scale=y[:, g : g + 1]
                )
            else:
                nc.scalar.activation(out=sg[:, g], in_=ps[:], func=AF.Sigmoid)
        gsum = spool.tile([P, G], f32)
        nc.vector.reduce_sum(out=gsum[:], in_=sg[:], axis=mybir.AxisListType.X)
        nc.vector.tensor_scalar_mul(gate_all[:, t], gsum[:], 1.0 / FD)
        s = spool.tile([P, G], f32)
        nc.vector.tensor_tensor(
            out=s[:], in0=gate_all[:, t], in1=att_all[:, t], op=A.mult
        )
        v["s"] = s

    def s5_out(t):
        v = st.pop(t)
        xp, s = v["xp"], v["s"]
        o = opool.tile([P, G, D], f32)
        for g in range(G):
            if g == 0:
                nc.vector.tensor_scalar(o[:, g], xp[:, g], s[:, g : g + 1], None, A.mult)
            else:
                nc.scalar.activation(
                    out=o[:, g], in_=xp[:, g], func=AF.Copy, scale=s[:, g : g + 1]
                )
        # SWDGE store: keeps SP free for transposes (mixing DMACopy with
        # DMATranspose on SP forces xbar-mode serialization), and the skew
        # keeps this store from head-of-line-blocking later loads
        nc.gpsimd.dma_start(out=out_v[t], in_=o[:])

    stages = (s0_load, s1_stats, s2_relu, s3_matmul, s4_sigmoid, s5_out)
    nstage = len(stages)
    for u in range(nit + nstage - 1):
        for k, stage in enumerate(stages):
            t = u - k
            if 0 <= t < nit:
                stage(t)

    nc.sync.dma_start(out=gt_v, in_=gate_all[:])


def _patch_tile_drain():
    """The walrus build in this container rejects Drain instructions that
    carry sync waits ("Too many sync wait commands").  Tile's kernel-tail
    drain stacks one wait per live semaphore lane onto a single Drain.
    Re-emit those waits as individual EventSemaphore waits (which walrus
    accepts) followed by a wait-free Drain — semantically identical."""
    import bass_rust
    import concourse.tile as tile
    from concourse.vector_clock import ScopedClock

    if getattr(tile.TileContext, "_drain_waits_patched", False):
        return

    def _drain_and_barrier(self, tick_clock, wait_clock):
        nc = self.nc
        probe = nc.sync.nop()
        wait_clock.add_sem_waits(probe.ins, ScopedClock({None: tick_clock.global_clock}))
        si = probe.ins.sync_info
        waits = list(si.on_wait) if si is not None else []
        probe.ins.sync_info = bass_rust.SyncInfo(on_wait=[], on_update=[])
        name_to_handle = {}
        assert self.sems is not None
        for h in self.sems.allocated().values():
            name_to_handle[h.name] = h
        for w in waits:
            h = name_to_handle.get(w.ant_name)
            if h is not None:
                nc.sync.wait_ge(h, w.wait_value)
            else:
                ev = nc.sync.nop()
                ev.ins.sync_info = bass_rust.SyncInfo(on_wait=[w], on_update=[])
        nc.sync.drain()
        nc.all_engine_barrier()
        popped = nc._tile_sem_poison_stack.pop()
        assert popped is self._sem_poison
        nc.clear_and_free_semaphores(list(self.sems.allocated().values()))
        nc.all_engine_barrier()

    tile.TileContext._drain_and_barrier = _drain_and_barrier
    tile.TileContext._drain_waits_patched = True


MAX_INST_WAITS = 2


def _patch_tile_wait_split():
    """Same walrus limitation as _patch_tile_drain, applied generally: any
    instruction carrying more than MAX_INST_WAITS sync waits gets the excess
    hoisted into standalone single-wait EventSemaphore instructions emitted
    just before it on the same engine."""
    import bass_rust
    import concourse.tile as tile
    from concourse import mybir

    if getattr(tile.TileContext, "_wait_split_patched", False):
        return
    orig = tile.TileContext._lower_ordered_insts

    def patched(self, ordered):
        nid = 0
        for bbname, insts in ordered.items():
            out = []
            for inst in insts:
                si = inst.sync_info
                waits = list(si.on_wait) if si else []
                n_up = len(si.on_update) if si else 0
                limit = max(0, MAX_INST_WAITS - n_up)
                if str(inst.opcode) == "Drain":
                    limit = 0
                if len(waits) > limit:
                    keep = waits[len(waits) - limit :] if limit else []
                    for w in waits[: len(waits) - limit]:
                        nid += 1
                        ev = mybir.InstEventSemaphore(name=f"WS{nid}-{inst.name}")
                        ev.engine = inst.engine
                        ev.sync_info = bass_rust.SyncInfo(on_wait=[w], on_update=[])
                        out.append(ev)
                    inst.sync_info = bass_rust.SyncInfo(
                        on_wait=keep, on_update=list(si.on_update) if si else []
                    )
                out.append(inst)
            ordered[bbname] = out
        return orig(self, ordered)

    tile.TileContext._lower_ordered_insts = patched
    tile.TileContext._wait_split_patched = True


def build_nc(n_rows=NL, fast=True):
    """Build the per-core Bass program (SPMD: same program on every core)."""
    from contextlib import ExitStack

    import concourse.bass as bass
    import concourse.tile as tile
    from concourse import mybir

    _patch_tile_drain()
    _patch_tile_wait_split()

    f32 = mybir.dt.float32
    bf16 = mybir.dt.bfloat16

    nc = bass.Bass("TRN2", target_bir_lowering=False, debug=False, num_devices=NCORES)
    unary = nc.dram_tensor("unary", [n_rows, D], f32, kind="ExternalInput").ap()
    pair = nc.dram_tensor("pair", [n_rows, D], f32, kind="ExternalInput").ap()
    attn = nc.dram_tensor("attn", [n_rows], f32, kind="ExternalInput").ap()
    wT = nc.dram_tensor("wT", [TWO_D, FD], bf16, kind="ExternalInput").ap()
    lnw = nc.dram_tensor("lnw", [TWO_D], f32, kind="ExternalInput").ap()
    lnb = nc.dram_tensor("lnb", [TWO_D], f32, kind="ExternalInput").ap()
    linb = nc.dram_tensor("linb", [FD], f32, kind="ExternalInput").ap()
    out = nc.dram_tensor("output", [n_rows, D], f32, kind="ExternalOutput").ap()
    gate = nc.dram_tensor("gate", [n_rows], f32, kind="ExternalOutput").ap()

    aps = (unary, pair, attn, wT, lnw, lnb, linb, out, gate)
    with tile.TileContext(nc) as tc:
        with ExitStack() as ctx:
            _emit(tc, ctx, aps, n_rows, fast)
    return nc


_CACHE = {}


def _get_nc(fast):
    key = ("nc", fast)
    if key not in _CACHE:
        _CACHE[key] = build_nc(NL, fast)
    return _CACHE[key]


def make_in_maps(unary_term, pair_term, attn_value, ln_weight, ln_bias,
                 lin_weight, lin_bias):
    wTb = np.ascontiguousarray(np.asarray(lin_weight).T).astype(ml_dtypes.bfloat16)
    unary_term = np.ascontiguousarray(np.asarray(unary_term, dtype=np.float32))
    pair_term = np.ascontiguousarray(np.asarray(pair_term, dtype=np.float32))
    attn_value = np.ascontiguousarray(np.asarray(attn_value, dtype=np.float32))
    common = {
        "wT": wTb,
        "lnw": np.asarray(ln_weight, dtype=np.float32),
        "lnb": np.asarray(ln_bias, dtype=np.float32),
        "linb": np.asarray(lin_bias, dtype=np.float32),
    }
    in_maps = []
    for i in range(NCORES):
        sl = slice(i * NL, (i + 1) * NL)
        in_maps.append(
            {
                "unary": unary_term[sl],
                "pair": pair_term[sl],
                "attn": attn_value[sl],
                **common,
            }
        )
    return in_maps


def kernel(unary_term, pair_term, attn_value, ln_weight, ln_bias,
           lin_weight, lin_bias):
    from concourse.bass_utils import run_bass_kernel_spmd

    fast = (
        np.allclose(np.asarray(ln_weight), 1.0)
        and np.allclose(np.asarray(ln_bias), 0.0)
        and np.allclose(np.asarray(lin_bias), 0.0)
    )
    nc = _get_nc(fast)
    in_maps = make_in_maps(
        unary_term, pair_term, attn_value, ln_weight, ln_bias, lin_weight, lin_bias
    )
    res = run_bass_kernel_spmd(nc, in_maps, list(range(NCORES)))
    output = np.concatenate([r["output"] for r in res.results], axis=0)
    gate = np.concatenate([r["gate"] for r in res.results], axis=0)
    return output.astype(np.float32), gate.astype(np.float32)


# revision 29
# speedup vs baseline: 1.2764x; 1.0417x over previous
"""Trainium2 Bass kernel for nn_MessagePassingUnit_v1 (gnn_message_passing).

Computation (per row r of N=131072):
    paired = concat(unary[r], pair[r])            # [1024]
    h = LayerNorm(paired) -> ReLU                 # [1024]
    z = h @ lin_weight.T + lin_bias               # [64]
    gate[r] = mean(sigmoid(z))                    # scalar
    output[r] = pair[r] * gate[r] * attn[r]       # [512]

Returns (output [N,512] f32, gate [N] f32).

Sharding: pure data parallel over N across 8 NeuronCores (rows split in 8
contiguous blocks); params replicated.

Per-core device pipeline (rows on partitions, 512 rows per iteration):
  DMA   : load unary/pair halves into one [128,4,1024] f32 tile + attn
  DVE   : bn_stats/bn_aggr -> mean/var; Newton-Raphson rsqrt (bit-trick seed)
  ACT   : h = Relu(x*r - mu*r) in one fused activation pass, output bf16
  DMA   : xbar transpose h -> hT (features on partitions), bf16
  PE    : z = sum_c hT_c.T @ WT_c accumulated in PSUM (bf16 x bf16 -> f32)
  ACT   : sigmoid(z) with free-dim accumulation -> sum of sigmoids per row
  DVE   : gate = accum/64 ; s = gate*attn
  GPSIMD: output = pair * s   (per-partition scalar multiply)
  DMA   : store output rows + gate
"""

import os
import sys

import numpy as np

for _p in ("/opt/trn_rl_repo", "/opt/pypackages"):
    if os.path.isdir(_p) and _p not in sys.path:
        sys.path.append(_p)

import ml_dtypes

N, D, FD = 131072, 512, 64
TWO_D = 2 * D
EPS = 1e-5
NCORES = 8
NL = N // NCORES          # rows per core
P = 128                   # partitions
G = 4                     # row-groups of 128 per iteration
RPI = P * G               # rows per iteration
NCH = TWO_D // P          # K chunks of 128 (8)
MAGIC = 0x5F3759DF        # fast inverse sqrt seed


def _emit(tc, ctx, aps, n_rows, fast):
    import concourse.bass as bass
    from concourse import mybir

    nc = tc.nc
    f32 = mybir.dt.float32
    bf16 = mybir.dt.bfloat16
    u32 = mybir.dt.uint32
    A = mybir.AluOpType
    AF = mybir.ActivationFunctionType

    unary, pair, attn, wT, lnw, lnb, linb, out, gate = aps
    nit = n_rows // RPI

    # p-major row mapping: row r = RPI*t + G*p + g, so each partition's G rows
    # are consecutive in DRAM -> 8KB-contiguous DMA descriptors.
    un_v = unary.rearrange("(t p g) d -> t p g d", p=P, g=G)
    pr_v = pair.rearrange("(t p g) d -> t p g d", p=P, g=G)
    at_v = attn.rearrange("(t p g) -> p t g", p=P, g=G)
    out_v = out.rearrange("(t p g) d -> t p g d", p=P, g=G)
    gt_v = gate.rearrange("(t p g) -> p t g", p=P, g=G)
    wt_v = wT.rearrange("(c p) f -> p c f", p=P)

    singles = ctx.enter_context(tc.tile_pool(name="singles", bufs=1))
    xupool = ctx.enter_context(tc.tile_pool(name="xu", bufs=4))
    xppool = ctx.enter_context(tc.tile_pool(name="xp", bufs=7 if fast else 4))
    hpool = ctx.enter_context(tc.tile_pool(name="h", bufs=3 if fast else 2))
    htpool = ctx.enter_context(tc.tile_pool(name="ht", bufs=3))
    opool = ctx.enter_context(tc.tile_pool(name="o", bufs=3))
    spool = ctx.enter_context(tc.tile_pool(name="small", bufs=6))
    sgpool = ctx.enter_context(tc.tile_pool(name="sg", bufs=4))
    pspool = ctx.enter_context(tc.tile_pool(name="ps", bufs=8, space="PSUM"))

    wt_sb = singles.tile([P, NCH, FD], bf16)
    nc.sync.dma_start(out=wt_sb[:], in_=wt_v)
    att_all = singles.tile([P, nit, G], f32)
    nc.sync.dma_start(out=att_all[:], in_=at_v)
    gate_all = singles.tile([P, nit, G], f32)

    if not fast:
        # broadcast LN affine params across partitions
        wb = singles.tile([P, TWO_D], f32)
        bb = singles.tile([P, TWO_D], f32)
        nc.gpsimd.dma_start(
            out=wb[:],
            in_=bass.AP(tensor=lnw.tensor, offset=lnw.offset, ap=[[0, P]] + list(lnw.ap)),
        )
        nc.gpsimd.dma_start(
            out=bb[:],
            in_=bass.AP(tensor=lnb.tensor, offset=lnb.offset, ap=[[0, P]] + list(lnb.ap)),
        )
        # lin_bias row for the K=1 bias matmul
        lb_sb = singles.tile([1, FD], f32)
        nc.sync.dma_start(out=lb_sb[:], in_=linb.rearrange("(o f) -> o f", o=1))
        one_sb = singles.tile([1, P], f32)
        nc.vector.memset(one_sb[:], 1.0)

    # Software-pipelined emission: the per-iteration dependency chain
    # (load -> stats -> relu -> transpose -> matmul -> sigmoid -> scale ->
    # store) is ~15-20us long while each engine only has ~8-10us of work per
    # iteration.  Each engine executes its queue in order, so emitting whole
    # iterations back-to-back serializes on the chain latency.  Emit in
    # skewed stages instead so every engine's stream interleaves several
    # iterations.
    st = {}

    def s0_load(t):
        xu = xupool.tile([P, G, D], f32)
        xp = xppool.tile([P, G, D], f32)
        nc.gpsimd.dma_start(out=xu[:], in_=un_v[t])
        nc.gpsimd.dma_start(out=xp[:], in_=pr_v[t])
        st[t] = {"xu": xu, "xp": xp}

    def s1_stats(t):
        v = st[t]
        xu, xp = v["xu"], v["xp"]
        st6 = spool.tile([P, G, 2, 6], f32)
        for g in range(G):
            nc.vector.bn_stats(out=st6[:, g, 0], in_=xu[:, g])
            nc.vector.bn_stats(out=st6[:, g, 1], in_=xp[:, g])
        mv = spool.tile([P, G, 2], f32)
        for g in range(G):
            nc.vector.bn_aggr(out=mv[:, g], in_=st6[:, g])

        # r = rsqrt(var + eps): bit-trick seed + 2 Newton iterations (DVE only)
        u = spool.tile([P, G], f32)
        nc.vector.tensor_scalar_add(u[:], mv[:, :, 1], EPS)
        y = spool.tile([P, G], f32)
        yu = y[:].bitcast(u32)
        uu = u[:].bitcast(u32)
        nc.vector.tensor_scalar(yu, uu, -0.5, float(MAGIC), A.mult, A.add)
        yy = spool.tile([P, G], f32)
        pp = spool.tile([P, G], f32)
        hh = spool.tile([P, G], f32)
        for _ in range(2):
            nc.vector.tensor_tensor(out=yy[:], in0=y[:], in1=y[:], op=A.mult)
            nc.vector.tensor_tensor(out=pp[:], in0=yy[:], in1=u[:], op=A.mult)
            nc.vector.tensor_scalar(hh[:], pp[:], -0.5, 1.5, A.mult, A.add)
            nc.vector.tensor_tensor(out=y[:], in0=y[:], in1=hh[:], op=A.mult)
        nmr = spool.tile([P, G], f32)
        nc.vector.tensor_scalar_mul(nmr[:], mv[:, :, 0], -1.0)
        v["y"] = y
        v["nmr"] = nmr

    def s2_relu(t):
        v = st[t]
        xu, xp, y, nmr = v["xu"], v["xp"], v["y"], v["nmr"]
        # fast path: h = relu(x - mu); the rsqrt scale r is folded into the
        # sigmoid's per-partition scale (r > 0 commutes with relu), keeping
        # the Newton-rsqrt chain off the critical path.
        h = hpool.tile([P, G, TWO_D], bf16)
        if fast:
            for g in range(G):
                if g < 2:
                    nc.vector.tensor_scalar(
                        h[:, g, 0:D], xu[:, g], nmr[:, g : g + 1], 0.0, A.add, A.max
                    )
                else:
                    nc.scalar.activation(
                        out=h[:, g, 0:D],
                        in_=xu[:, g],
                        func=AF.Relu,
                        bias=nmr[:, g : g + 1],
                    )
                nc.scalar.activation(
                    out=h[:, g, D:TWO_D],
                    in_=xp[:, g],
                    func=AF.Relu,
                    bias=nmr[:, g : g + 1],
                )
        else:
            nmr2 = spool.tile([P, G], f32)
            nc.vector.tensor_tensor(out=nmr2[:], in0=nmr[:], in1=y[:], op=A.mult)
            tmp = hpool.tile([P, G, TWO_D], f32, tag="tmp_general")
            for g in range(G):
                nc.scalar.activation(
                    out=tmp[:, g, 0:D],
                    in_=xu[:, g],
                    func=AF.Identity,
                    bias=nmr2[:, g : g + 1],
                    scale=y[:, g : g + 1],
                )
                nc.scalar.activation(
                    out=tmp[:, g, D:TWO_D],
                    in_=xp[:, g],
                    func=AF.Identity,
                    bias=nmr2[:, g : g + 1],
                    scale=y[:, g : g + 1],
                )
                nc.vector.tensor_tensor(out=tmp[:, g], in0=tmp[:, g], in1=wb[:], op=A.mult)
                nc.vector.tensor_tensor(out=tmp[:, g], in0=tmp[:, g], in1=bb[:], op=A.add)
                nc.scalar.activation(out=h[:, g], in_=tmp[:, g], func=AF.Relu)

        v["h"] = h

    def s3_transpose(t):
        # transpose h: [128 rows, (g d)] -> [128 feat, (g c), 128 rows]
        # The xbar transpose serializes against ALL other DMA traffic
        # (hardware xbar-mode transition guard), so give it its own pipeline
        # stage: by the time the DMA subsystem switches into transpose mode,
        # h has been ready for a full period and the payload fires
        # immediately.
        v = st[t]
        h = v.pop("h")
        hT = htpool.tile([P, G * NCH, P], bf16)
        nc.sync.dma_start(
            out=hT[:], in_=h[:].rearrange("p g d -> p (g d)"), transpose=True
        )
        v["hT"] = hT

    def s4_matmul(t):
        v = st[t]
        hT = v["hT"]
        pss = []
        for g in range(G):
            ps = pspool.tile([P, FD], f32)
            for c in range(NCH):
                nc.tensor.matmul(
                    ps[:],
                    hT[:, g * NCH + c],
                    wt_sb[:, c],
                    start=(c == 0),
                    stop=(fast and c == NCH - 1),
                )
            if not fast:
                nc.tensor.matmul(ps[:], one_sb[:], lb_sb[:], start=False, stop=True)
            pss.append(ps)
        v["pss"] = pss

    def s4_sigmoid(t):
        v = st[t]
        y = v["y"]
        sg = sgpool.tile([P, G, FD], f32)
        for g, ps in enumerate(v.pop("pss")):
            if fast:
                nc.scalar.activation(
                    out=sg[:, g], in_=ps[:], func=AF.Sigmoid, scale=y[:, g : g + 1]
                )
            else:
                nc.scalar.activation(out=sg[:, g], in_=ps[:], func=AF.Sigmoid)
        gsum = spool.tile([P, G], f32)
        nc.vector.reduce_sum(out=gsum[:], in_=sg[:], axis=mybir.AxisListType.X)
        nc.vector.tensor_scalar_mul(gate_all[:, t], gsum[:], 1.0 / FD)
        s = spool.tile([P, G], f32)
        nc.vector.tensor_tensor(
            out=s[:], in0=gate_all[:, t], in1=att_all[:, t], op=A.mult
        )
        v["s"] = s

    def s5_out(t):
        v = st.pop(t)
        xp, s = v["xp"], v["s"]
        o = opool.tile([P, G, D], f32)
        for g in range(G):
            if g == 0:
                nc.vector.tensor_scalar(o[:, g], xp[:, g], s[:, g : g + 1], None, A.mult)
            else:
                nc.scalar.activation(
                    out=o[:, g], in_=xp[:, g], func=AF.Copy, scale=s[:, g : g + 1]
                )
        # SWDGE store: keeps SP free for transposes (mixing DMACopy with
        # DMATranspose on SP forces xbar-mode serialization), and the skew
        # keeps this store from head-of-line-blocking later loads
        nc.gpsimd.dma_start(out=out_v[t], in_=o[:])

    stages = (s0_load, s1_stats, s2_relu, s3_transpose, s4_matmul, s4_sigmoid, s5_out)
    nstage = len(stages)
    for u in range(nit + nstage - 1):
        for k, stage in enumerate(stages):
            t = u - k
            if 0 <= t < nit:
                stage(t)

    nc.sync.dma_start(out=gt_v, in_=gate_all[:])


def _patch_tile_drain():
    """The walrus build in this container rejects Drain instructions that
    carry sync waits ("Too many sync wait commands").  Tile's kernel-tail
    drain stacks one wait per live semaphore lane onto a single Drain.
    Re-emit those waits as individual EventSemaphore waits (which walrus
    accepts) followed by a wait-free Drain — semantically identical."""
    import bass_rust
    import concourse.tile as tile
    from concourse.vector_clock import ScopedClock

    if getattr(tile.TileContext, "_drain_waits_patched", False):
        return

    def _drain_and_barrier(self, tick_clock, wait_clock):
        nc = self.nc
        probe = nc.sync.nop()
        wait_clock.add_sem_waits(probe.ins, ScopedClock({None: tick_clock.global_clock}))
        si = probe.ins.sync_info
        waits = list(si.on_wait) if si is not None else []
        probe.ins.sync_info = bass_rust.SyncInfo(on_wait=[], on_update=[])
        name_to_handle = {}
        assert self.sems is not None
        for h in self.sems.allocated().values():
            name_to_handle[h.name] = h
        for w in waits:
            h = name_to_handle.get(w.ant_name)
            if h is not None:
                nc.sync.wait_ge(h, w.wait_value)
            else:
                ev = nc.sync.nop()
                ev.ins.sync_info = bass_rust.SyncInfo(on_wait=[w], on_update=[])
        nc.sync.drain()
        nc.all_engine_barrier()
        popped = nc._tile_sem_poison_stack.pop()
        assert popped is self._sem_poison
        nc.clear_and_free_semaphores(list(self.sems.allocated().values()))
        nc.all_engine_barrier()

    tile.TileContext._drain_and_barrier = _drain_and_barrier
    tile.TileContext._drain_waits_patched = True


MAX_INST_WAITS = 2


def _patch_tile_wait_split():
    """Same walrus limitation as _patch_tile_drain, applied generally: any
    instruction carrying more than MAX_INST_WAITS sync waits gets the excess
    hoisted into standalone single-wait EventSemaphore instructions emitted
    just before it on the same engine."""
    import bass_rust
    import concourse.tile as tile
    from concourse import mybir

    if getattr(tile.TileContext, "_wait_split_patched", False):
        return
    orig = tile.TileContext._lower_ordered_insts

    def patched(self, ordered):
        nid = 0
        for bbname, insts in ordered.items():
            out = []
            for inst in insts:
                si = inst.sync_info
                waits = list(si.on_wait) if si else []
                n_up = len(si.on_update) if si else 0
                limit = max(0, MAX_INST_WAITS - n_up)
                if str(inst.opcode) == "Drain":
                    limit = 0
                if len(waits) > limit:
                    keep = waits[len(waits) - limit :] if limit else []
                    for w in waits[: len(waits) - limit]:
                        nid += 1
                        ev = mybir.InstEventSemaphore(name=f"WS{nid}-{inst.name}")
                        ev.engine = inst.engine
                        ev.sync_info = bass_rust.SyncInfo(on_wait=[w], on_update=[])
                        out.append(ev)
                    inst.sync_info = bass_rust.SyncInfo(
                        on_wait=keep, on_update=list(si.on_update) if si else []
                    )
                out.append(inst)
            ordered[bbname] = out
        return orig(self, ordered)

    tile.TileContext._lower_ordered_insts = patched
    tile.TileContext._wait_split_patched = True


def build_nc(n_rows=NL, fast=True):
    """Build the per-core Bass program (SPMD: same program on every core)."""
    from contextlib import ExitStack

    import concourse.bass as bass
    import concourse.tile as tile
    from concourse import mybir

    _patch_tile_drain()
    _patch_tile_wait_split()

    f32 = mybir.dt.float32
    bf16 = mybir.dt.bfloat16

    nc = bass.Bass("TRN2", target_bir_lowering=False, debug=False, num_devices=NCORES)
    unary = nc.dram_tensor("unary", [n_rows, D], f32, kind="ExternalInput").ap()
    pair = nc.dram_tensor("pair", [n_rows, D], f32, kind="ExternalInput").ap()
    attn = nc.dram_tensor("attn", [n_rows], f32, kind="ExternalInput").ap()
    wT = nc.dram_tensor("wT", [TWO_D, FD], bf16, kind="ExternalInput").ap()
    lnw = nc.dram_tensor("lnw", [TWO_D], f32, kind="ExternalInput").ap()
    lnb = nc.dram_tensor("lnb", [TWO_D], f32, kind="ExternalInput").ap()
    linb = nc.dram_tensor("linb", [FD], f32, kind="ExternalInput").ap()
    out = nc.dram_tensor("output", [n_rows, D], f32, kind="ExternalOutput").ap()
    gate = nc.dram_tensor("gate", [n_rows], f32, kind="ExternalOutput").ap()

    aps = (unary, pair, attn, wT, lnw, lnb, linb, out, gate)
    with tile.TileContext(nc) as tc:
        with ExitStack() as ctx:
            _emit(tc, ctx, aps, n_rows, fast)
    return nc


_CACHE = {}


def _get_nc(fast):
    key = ("nc", fast)
    if key not in _CACHE:
        _CACHE[key] = build_nc(NL, fast)
    return _CACHE[key]


def make_in_maps(unary_term, pair_term, attn_value, ln_weight, ln_bias,
                 lin_weight, lin_bias):
    wTb = np.ascontiguousarray(np.asarray(lin_weight).T).astype(ml_dtypes.bfloat16)
    unary_term = np.ascontiguousarray(np.asarray(unary_term, dtype=np.float32))
    pair_term = np.ascontiguousarray(np.asarray(pair_term, dtype=np.float32))
    attn_value = np.ascontiguousarray(np.asarray(attn_value, dtype=np.float32))
    common = {
        "wT": wTb,
        "lnw": np.asarray(ln_weight, dtype=np.float32),
        "lnb": np.asarray(ln_bias, dtype=np.float32),
        "linb": np.asarray(lin_bias, dtype=np.float32),
    }
    in_maps = []
    for i in range(NCORES):
        sl = slice(i * NL, (i + 1) * NL)
        in_maps.append(
            {
                "unary": unary_term[sl],
                "pair": pair_term[sl],
                "attn": attn_value[sl],
                **common,
            }
        )
    return in_maps


def kernel(unary_term, pair_term, attn_value, ln_weight, ln_bias,
           lin_weight, lin_bias):
    from concourse.bass_utils import run_bass_kernel_spmd

    fast = (
        np.allclose(np.asarray(ln_weight), 1.0)
        and np.allclose(np.asarray(ln_bias), 0.0)
        and np.allclose(np.asarray(lin_bias), 0.0)
    )
    nc = _get_nc(fast)
    in_maps = make_in_maps(
        unary_term, pair_term, attn_value, ln_weight, ln_bias, lin_weight, lin_bias
    )
    res = run_bass_kernel_spmd(nc, in_maps, list(range(NCORES)))
    output = np.concatenate([r["output"] for r in res.results], axis=0)
    gate = np.concatenate([r["gate"] for r in res.results], axis=0)
    return output.astype(np.float32), gate.astype(np.float32)
